# revision 3
# baseline (speedup 1.0000x reference)
"""GCMC GraphConv kernel for 8 Trainium2 NeuronCores.

Computation:  out = ci * segment_sum((input_feat @ weight * cj)[src], dst)

Strategy v2 (aggregate-then-transform, dst-sharded, no collectives):
  - Algebra: out = ci * (A^T (cj*X)) W  — the dense transform W commutes with
    the segment-sum, so we aggregate raw (cj-scaled) X rows per destination
    and multiply by W once per dst block at the end.  This removes the
    X@W pre-pass and its HBM h round-trip entirely; the per-edge gather
    reads bf16 X' rows (256B, all 128 input features) straight from HBM.
  - Nodes are 1D-partitioned by destination: core c owns 98 dst blocks of
    128 rows, assigned by sorted edge count for balance.  Each edge is
    routed (on host) to the core owning its destination.
  - The per-edge gather uses SWDGE dma_gather with queue_num cycling over
    4 queues: calls on different queues run concurrently on different
    GPSIMD Q7 core pairs (~2.2ns/desc aggregate vs 6.35ns serial).
  - Scatter is a one-hot matmul accumulated TRANSPOSED:
        psT[feat, dst] += msg[e, feat]^T-contract oh[e, dst]
    (lhsT=msg, rhs=onehot), flushed into accT [128 feat, 98*128 dst] f32.
    One-hots are built in batches (one DVE is_equal per gather chunk).
  - Epilogue per dst block: matmul(lhsT=accT block, rhs=W) -> [128 dst, 64],
    scale by ci, store.
  - Edges are host-sorted by (src window of 25000, dst block); per-(w,slot)
    groups are padded to the max over cores (shared SPMD program); pad
    edges gather row 0 and carry dstb = -1 (no one-hot match -> zero).
"""

import dataclasses
import math

import numpy as np
import ml_dtypes

import concourse.bacc as bacc
import concourse.mybir as mybir
import concourse.tile as tile
from concourse.bass_utils import run_bass_kernel_spmd

BF16 = ml_dtypes.bfloat16
P = 128
NCORES = 8
D_IN = 128


@dataclasses.dataclass(frozen=True)
class Cfg:
    N: int = 100000
    D_OUT: int = 64
    NWIN: int = 4            # src windows; N/NWIN must be < 32768 (int16 idx)
    MAX_CHUNK_TILES: int = 8   # gather chunk (1024 descs = SWDGE ring capacity)
    NQUEUES: int = 4         # SWDGE queues; round-robin -> concurrent desc-gen
    SCRATCH: int = 32768     # dynamic DMA descriptor carveout (bytes/partition)

    @property
    def n_loc(self):
        return self.N // NCORES

    @property
    def nblk(self):
        return math.ceil(self.n_loc / P)

    @property
    def win(self):
        return self.N // self.NWIN


CFG = Cfg()


# ---------------------------------------------------------------- host prep

def shard_edges(cfg: Cfg, src, dst):
    """Route and sort edges; build per-core padded index/dst arrays.

    Destination blocks are assigned to (core, slot) pairs by sorted edge
    count so each slot's 8 blocks have similar counts (the SPMD program
    pads every (w, slot) group to the max over its 8 cores).

    Returns (G, per_core, block_of):
      G[w][s]       tiles of (window w, slot s) — identical across cores
      per_core[c]   dict with idx{w} / dstb{w} device arrays
      block_of[c,s] global dst block handled by core c, slot s
    """
    nblk, win, nw_ = cfg.nblk, cfg.win, cfg.NWIN
    src = np.asarray(src, dtype=np.int64)
    dst = np.asarray(dst, dtype=np.int64)
    gb = dst >> 7                            # global dst block
    dstb = (dst & 127).astype(np.float32)    # dst within block
    wine = src // win
    src_loc = (src - wine * win).astype(np.int16)

    nblk_g = NCORES * nblk                   # padded global block count
    bc = np.bincount(gb, minlength=nblk_g)   # edges per global block
    order = np.argsort(-bc, kind="stable")   # blocks by descending count
    block_of = np.empty((NCORES, nblk), dtype=np.int64)
    block_core = np.empty(nblk_g, dtype=np.int64)
    block_slot = np.empty(nblk_g, dtype=np.int64)
    for s in range(nblk):
        grp = order[s * NCORES:(s + 1) * NCORES]
        block_of[:, s] = grp
        block_core[grp] = np.arange(NCORES)
        block_slot[grp] = s

    core = block_core[gb]
    slot = block_slot[gb]

    gid = (core * nw_ + wine) * nblk + slot
    counts = np.bincount(gid, minlength=NCORES * nw_ * nblk)
    counts = counts.reshape(NCORES, nw_, nblk)
    G = -(-counts.max(axis=0) // P)          # ceil tiles per (w, slot)
    tiles_w = G.sum(axis=1)                  # [NWIN]

    off_ws = np.zeros((nw_, nblk), dtype=np.int64)
    off_ws[:, 1:] = np.cumsum(G[:, :-1], axis=1) * P

    per_core = []
    for c in range(NCORES):
        m = core == c
        sl, db, we, bl = src_loc[m], dstb[m], wine[m], slot[m]
        # sort by (window, slot, src) — src-ordered within group for DRAM
        # row locality in the gather
        key = (we * nblk + bl) * (win + 1) + sl
        o = np.argsort(key, kind="stable")
        ks = (we[o] * nblk + bl[o])
        gcnt = np.bincount(ks, minlength=nw_ * nblk)
        gstart = np.concatenate([[0], np.cumsum(gcnt)[:-1]])
        within = np.arange(ks.size) - gstart[ks]
        wsel, ssel = ks // nblk, ks % nblk
        pos = off_ws[wsel, ssel] + within
        maps = {}
        for w in range(nw_):
            nw_edges = int(tiles_w[w]) * P
            ia = np.zeros(nw_edges, dtype=np.int16)         # pad -> row 0
            da = np.full(nw_edges, -1.0, dtype=np.float32)  # pad -> no match
            sel = wsel == w
            ia[pos[sel]] = sl[o][sel]
            da[pos[sel]] = db[o][sel]
            maps[f"idx{w}"] = np.ascontiguousarray(
                np.tile(ia.reshape(-1, 16).T, (8, 1)))
            maps[f"dstb{w}"] = np.ascontiguousarray(da.reshape(-1, P).T)
        per_core.append(maps)
    return G, per_core, block_of


def host_inputs(cfg: Cfg, input_feat, weight, cj, ci, block_of):
    """Shared (replicated) device inputs + per-core civ (slot layout)."""
    N, nblk = cfg.N, cfg.nblk
    # X' = cj * X, node-major bf16 (256B rows = dma_gather granularity)
    xs = (np.asarray(input_feat, dtype=np.float32)
          * np.asarray(cj, dtype=np.float32)).astype(BF16)
    xw = np.ascontiguousarray(xs)
    wgt = np.ascontiguousarray(np.asarray(weight, dtype=np.float32))
    iot = np.ascontiguousarray(
        np.broadcast_to(np.arange(P, dtype=np.float32), (P, P)).astype(BF16))
    # ci in (core, slot) layout: civ[c][p, s] = ci[block_of[c,s]*128 + p]
    cip = np.zeros(NCORES * nblk * P, dtype=np.float32)
    cif = np.asarray(ci, dtype=np.float32).reshape(-1)
    cip[:N] = cif
    cip = cip.reshape(NCORES * nblk, P)
    civs = [np.ascontiguousarray(cip[block_of[c]].T) for c in range(NCORES)]
    return {"xw": xw, "wgt": wgt, "iot": iot}, civs


# ---------------------------------------------------------------- device IR

def tile_blocks(cfg: Cfg, G, w):
    """Per-tile (block, k, g) for window w, in edge order."""
    out = []
    for b in range(cfg.nblk):
        g = int(G[w][b])
        for k in range(g):
            out.append((b, k, g))
    return out


def build_nc(cfg: Cfg, G):
    f32, bf16, i16 = mybir.dt.float32, mybir.dt.bfloat16, mybir.dt.int16
    dout, nblk, win = cfg.D_OUT, cfg.nblk, cfg.win
    tiles_w = [int(sum(G[w])) for w in range(cfg.NWIN)]

    nc = bacc.Bacc("TRN2", target_bir_lowering=False, debug=False,
                   num_swdge_queues=cfg.NQUEUES,
                   dynamic_dma_scratch_size=cfg.SCRATCH)
    xw = nc.dram_tensor("xw", [cfg.N, D_IN], bf16, kind="ExternalInput")
    wgt = nc.dram_tensor("wgt", [D_IN, dout], f32, kind="ExternalInput")
    civ = nc.dram_tensor("civ", [P, nblk], f32, kind="ExternalInput")
    iot = nc.dram_tensor("iot", [P, P], bf16, kind="ExternalInput")
    idx_t = [nc.dram_tensor(f"idx{w}", [P, tiles_w[w] * 8], i16,
                            kind="ExternalInput") for w in range(cfg.NWIN)]
    dstb_t = [nc.dram_tensor(f"dstb{w}", [P, tiles_w[w]], f32,
                             kind="ExternalInput") for w in range(cfg.NWIN)]
    out_t = nc.dram_tensor("out", [nblk * P, dout], f32, kind="ExternalOutput")

    with tile.TileContext(nc) as tc:
        with (
            tc.tile_pool(name="const", bufs=1) as cpool,
            tc.tile_pool(name="idx", bufs=2) as ipool,
            tc.tile_pool(name="msg", bufs=8) as mpool,
            tc.tile_pool(name="oh", bufs=8) as opool,
            tc.tile_pool(name="ps", bufs=4, space="PSUM") as pspool,
            tc.tile_pool(name="pso", bufs=4, space="PSUM") as psopool,
            tc.tile_pool(name="acc", bufs=1) as apool,
        ):
            wgt_sb = cpool.tile([P, dout], f32, tag="wgt")
            nc.sync.dma_start(out=wgt_sb[:], in_=wgt[:])
            iota_sb = cpool.tile([P, P], bf16, tag="iot")
            nc.sync.dma_start(out=iota_sb[:], in_=iot[:])
            ci_sb = cpool.tile([P, nblk], f32, tag="ci")
            nc.sync.dma_start(out=ci_sb[:], in_=civ[:])
            accT = apool.tile([P, nblk * P], f32, tag="accT")
            nc.vector.memset(accT[:], 0.0)
            out_sb = apool.tile([P, nblk * dout], f32, tag="out")

            st = {"ps": None}

            def emit_chunk(w, t0, tb, idx_sb, dst_sb, qn):
                """Gather one chunk of edges and matmul-scatter it."""
                t1 = min(t0 + cfg.MAX_CHUNK_TILES, len(tb))
                nt = t1 - t0
                ne = nt * P
                msg = mpool.tile([P, nt * D_IN], bf16, tag="msg")
                nc.gpsimd.dma_gather(
                    msg[:].rearrange("p (t f) -> p t f", f=D_IN),
                    xw[w * win:(w + 1) * win, :],
                    idx_sb[:, t0 * 8:t1 * 8],
                    ne, ne, D_IN,
                    queue_num=qn)
                oh = opool.tile([P, nt * P], bf16, tag="oh")
                nc.vector.tensor_tensor(
                    out=oh[:].rearrange("p (t n) -> p t n", n=P),
                    in0=dst_sb[:, t0:t1].rearrange("p (t o) -> p t o", o=1)
                        .to_broadcast([P, nt, P]),
                    in1=iota_sb[:].rearrange("p (o n) -> p o n", o=1)
                        .to_broadcast([P, nt, P]),
                    op=mybir.AluOpType.is_equal)
                for t in range(t0, t1):
                    b, k, g = tb[t]
                    if k == 0:
                        st["ps"] = pspool.tile([P, P], f32, tag="psT",
                                               name="psT")
                    ps = st["ps"]
                    nc.tensor.matmul(
                        out=ps[:],
                        lhsT=msg[:, (t - t0) * D_IN:(t - t0 + 1) * D_IN],
                        rhs=oh[:, (t - t0) * P:(t - t0 + 1) * P],
                        start=(k == 0), stop=(k == g - 1))
                    if k == g - 1:
                        nc.vector.tensor_add(
                            out=accT[:, b * P:(b + 1) * P],
                            in0=accT[:, b * P:(b + 1) * P],
                            in1=ps[:])

            qn = 0
            for w in range(cfg.NWIN):
                idx_sb = ipool.tile([P, tiles_w[w] * 8], i16, tag="idx")
                nc.sync.dma_start(out=idx_sb[:], in_=idx_t[w][:])
                dst_sb = ipool.tile([P, tiles_w[w]], f32, tag="dstb")
                nc.sync.dma_start(out=dst_sb[:], in_=dstb_t[w][:])

                tb = tile_blocks(cfg, G, w)
                for t0 in range(0, len(tb), cfg.MAX_CHUNK_TILES):
                    emit_chunk(w, t0, tb, idx_sb, dst_sb, qn)
                    qn = (qn + 1) % cfg.NQUEUES

            # ---- epilogue: out_b = (accT_b)^T @ W, scale by ci, store ----
            for b in range(nblk):
                pso = psopool.tile([P, dout], f32, tag="pso")
                nc.tensor.matmul(
                    out=pso[:],
                    lhsT=accT[:, b * P:(b + 1) * P],
                    rhs=wgt_sb[:],
                    start=True, stop=True)
                nc.scalar.mul(
                    out_sb[:, b * dout:(b + 1) * dout],
                    pso[:],
                    ci_sb[:, b:b + 1])
            nc.sync.dma_start(
                out=out_t[:].rearrange("(b p) f -> p b f", p=P),
                in_=out_sb[:].rearrange("p (b f) -> p b f", f=dout))
    nc.compile()
    return nc


# ---------------------------------------------------------------- entry

def run(cfg: Cfg, input_feat, weight, cj, ci, src_idx, dst_idx, **run_kwargs):
    G, per_core, block_of = shard_edges(cfg, src_idx, dst_idx)
    shared, civs = host_inputs(cfg, input_feat, weight, cj, ci, block_of)
    nc = build_nc(cfg, G)
    in_maps = []
    for c in range(NCORES):
        m = dict(shared)
        m["civ"] = civs[c]
        m.update(per_core[c])
        in_maps.append(m)
    res = run_bass_kernel_spmd(nc, in_maps, core_ids=list(range(NCORES)),
                               **run_kwargs)
    # un-permute: core c slot s holds global dst block block_of[c, s]
    full = np.zeros((NCORES * cfg.nblk * P, cfg.D_OUT), dtype=np.float32)
    blk_rows = full.reshape(NCORES * cfg.nblk, P, cfg.D_OUT)
    for c in range(NCORES):
        o = res.results[c]["out"].reshape(cfg.nblk, P, cfg.D_OUT)
        blk_rows[block_of[c]] = o
    return full[:cfg.N], res


def kernel(input_feat, weight, cj, ci, src_idx, dst_idx):
    out, _ = run(CFG, input_feat, weight, cj, ci, src_idx, dst_idx)
    return out
